# revision 49
# baseline (speedup 1.0000x reference)
"""Multi-head self-attention (B=4,S=2048,D=1024,H=16,DH=64, causal) on 8 trn2 cores.

Sharding: core c -> batch b=c//2, head-group g=c%2 (8 heads each).

~289us (from the 314us baseline; runs vary +-10% with the chip power
state: P0 downclocks the PE 2.4->2.0GHz, visible as min matmul dur
258ns instead of 215ns in the trace).  What changed vs baseline:
- Causal masking moved OFF the PE: exp runs unmasked on the diagonal
  strips, then one strided DVE multiply zeroes above-diagonal (keep-mask
  ltB2).  Saves ~14us of mask matmuls on the bottleneck engine.
- DMA head: only 3 DMA rings exist (gpsimd=SW-DGE ~1us/descriptor+slow
  start; sync/scalar=HW-DGE live at ~2.5us).  The 16 DMA engines drain
  all rings round-robin per descriptor, so ring ORDER is the only
  latency lever: xb0+wq f-chunks first on the two HW rings, wk/wv/wp
  behind, x-blocks 1-3 at ring tails.  gpsimd's queue carries NO DMAs:
  it is strict FIFO and the Tile scheduler can hoist a sem-gated DMA
  ahead of latency-critical renorm PartitionBroadcasts (cost 8us when
  it happened).
- Emission engine: B(sb) projection chains split into 4-matmul half
  pieces + proj jt pieces, held in a deque, injected 1-3 per attention
  u-round (cfg per qb with post-boundary skip slots) and force-drained
  before each dependent head-pair.  Keeps PE dense while ACT (exp)
  paces the attention stream.
- Lazy ot PSUM allocation (first attn@V, not pair start) avoids
  ot-pool WAR stalls at head-pair boundaries.
- qb=3 tail: proj(3) for jt0/jt1 pre-runs co=0..2 during hp3's
  attention; only co=3 waits for the last renorm.  Tail bias-adds on
  ACT (idle there) instead of DVE (tail bottleneck).
- Scores/AV stay bf16: fp8 e4m3 fails numerics everywhere (3e-2..8e-2
  vs the 2e-2 gate, measured in numpy), and fp8 DoubleRow is the only
  fp8 speed win.  PE floor is output-port-bound: scores+AV 116us +
  projections 110us per core.
- exp on ACT batched over k-tile pairs ([128,1024] two-bank PSUM reads).
- attn@V with ones-column denominator (M=65); renorm via DVE copy +
  reciprocal_approx_fast + gpsimd partition_broadcast + DVE mult.
  (reciprocal directly from the PSUM row at partition 64 computes
  garbage - don't.)
- bv and bp folded host-side into the output-projection bias.

K-projection quirk (reference views k as (B,S,DH,H)): head h uses Wk rows
[dh*16+h for dh in range(64)] -- handled by host-side row gather.
"""
import numpy as np
import ml_dtypes
from collections import deque

import concourse.mybir as mybir
import concourse.tile as tile
from concourse import bacc
from concourse.bass_utils import run_bass_kernel_spmd

F32 = mybir.dt.float32
BF16 = mybir.dt.bfloat16
AF = mybir.ActivationFunctionType
ALU = mybir.AluOpType

B, S, D, H, DH = 4, 2048, 1024, 16, 64
FG = 512          # features per head-group (8 heads * 64)
N_CORES = 8
SCALE = 0.125     # 1/sqrt(64)

_NC = None


def _build():
    nc = bacc.Bacc("TRN2", target_bir_lowering=False, debug=False,
                   num_devices=N_CORES, enable_asserts=False)
    xbT_d = nc.dram_tensor("xbT", [D, S], BF16, kind="ExternalInput").ap()
    wqT_d = nc.dram_tensor("wqT", [D, FG], BF16, kind="ExternalInput").ap()
    wkT_d = nc.dram_tensor("wkT", [D, FG], BF16, kind="ExternalInput").ap()
    wvT_d = nc.dram_tensor("wvT", [D, FG], BF16, kind="ExternalInput").ap()
    wpT_d = nc.dram_tensor("wpT", [FG, D], BF16, kind="ExternalInput").ap()
    bqs_d = nc.dram_tensor("bqs", [128, 4], F32, kind="ExternalInput").ap()
    bks_d = nc.dram_tensor("bks", [128, 4], F32, kind="ExternalInput").ap()
    bps_d = nc.dram_tensor("bps", [128, 8], F32, kind="ExternalInput").ap()
    ltB2_d = nc.dram_tensor("ltB2", [128, 256], BF16, kind="ExternalInput").ap()
    out_d = nc.dram_tensor("outT", [D, S], F32, kind="ExternalOutput").ap()

    with tile.TileContext(nc) as tc:
        with tc.tile_pool(name="persist", bufs=1) as pp, \
             tc.tile_pool(name="xin", bufs=3) as xp, \
             tc.tile_pool(name="etile", bufs=8) as ep, \
             tc.tile_pool(name="small", bufs=4) as sp, \
             tc.tile_pool(name="outtile", bufs=4) as op, \
             tc.tile_pool(name="pspair", bufs=3, space="PSUM") as ps_pair, \
             tc.tile_pool(name="psot", bufs=2, space="PSUM") as ps_ot:

            # ---- persistent SBUF tensors ----
            wq = pp.tile([128, 8, FG], BF16)   # [dp, do, f]  (pre-scaled 1/8)
            wk = pp.tile([128, 8, FG], BF16)
            wv = pp.tile([128, 8, FG], BF16)
            wp = pp.tile([128, 4, D], BF16)    # [cp, co, j]
            qt = pp.tile([128, 4, S], BF16)    # [fp, fo, s]
            kt = pp.tile([128, 4, S], BF16)
            va = pp.tile([128, 16, 8, DH + 1], BF16)  # [skp, sko, h, dh|1]
            on_ = pp.tile([128, 4, S], BF16)   # renormed out^T  [cp, co, s]
            ltB2 = pp.tile([128, 256], BF16)
            bqs = pp.tile([128, 4], F32)
            bks = pp.tile([128, 4], F32)
            bps = pp.tile([128, 8], F32)

            xbT_r = xbT_d.rearrange("(do dp) s -> dp do s", dp=128)
            wq_r = wqT_d.rearrange("(do dp) f -> dp do f", dp=128)
            wk_r = wkT_d.rearrange("(do dp) f -> dp do f", dp=128)
            wv_r = wvT_d.rearrange("(do dp) f -> dp do f", dp=128)
            wp_r = wpT_d.rearrange("(co cp) j -> cp co j", cp=128)
            xtiles = {}

            def emit_xdma(sb):
                # always on the two HW rings: the gpsimd queue must stay
                # free of DMAs (it is strict FIFO and the scheduler may
                # hoist a sem-gated DMA ahead of the latency-critical
                # renorm PartitionBroadcasts)
                xb = xp.tile([128, 8, 512], BF16)
                ssl = slice(sb * 512, (sb + 1) * 512)
                nc.sync.dma_start(xb[:, 0:4, :], xbT_r[:, 0:4, ssl])
                nc.scalar.dma_start(xb[:, 4:8, :], xbT_r[:, 4:8, ssl])
                xtiles[sb] = xb

            # only 3 DMA queues exist: gpsimd (software DGE; slow to start,
            # ~1us per descriptor batch), sync(SP) and scalar(Activation)
            # (hardware DGE, live at ~2.5us).  The 16 DMA engines drain all
            # rings round-robin at descriptor granularity, so aggregate
            # bandwidth (~440GB/s) is shared by whatever is in flight: the
            # lever for latency is ring ORDER.  Critical prefix xb0+wq
            # first on the two HW rings, then wk, wv, wp; the gpsimd ring
            # only prefetches xb1 (xb2/xb3 deferred by xin bufs=2
            # back-pressure).
            nc.scalar.dma_start(bqs[:], bqs_d[:])
            nc.scalar.dma_start(bks[:], bks_d[:])
            nc.scalar.dma_start(ltB2[:], ltB2_d[:])
            nc.scalar.dma_start(bps[:], bps_d[:])
            emit_xdma(0)
            # wq/wk f-sliced so each Q/K chain's weight block arrives
            # just-in-time behind xb0
            nc.sync.dma_start(wq[:, :, 0:128], wq_r[:, :, 0:128])
            nc.scalar.dma_start(wq[:, :, 128:256], wq_r[:, :, 128:256])
            nc.sync.dma_start(wq[:, :, 256:384], wq_r[:, :, 256:384])
            nc.scalar.dma_start(wq[:, :, 384:512], wq_r[:, :, 384:512])
            nc.sync.dma_start(wk[:, :, 0:128], wk_r[:, :, 0:128])
            nc.scalar.dma_start(wv[:, 4:8, :], wv_r[:, 4:8, :])
            nc.sync.dma_start(wv[:, 0:4, :], wv_r[:, 0:4, :])
            nc.scalar.dma_start(wk[:, :, 128:256], wk_r[:, :, 128:256])
            nc.sync.dma_start(wk[:, :, 256:384], wk_r[:, :, 256:384])
            nc.scalar.dma_start(wk[:, :, 384:512], wk_r[:, :, 384:512])
            nc.sync.dma_start(wp[:, 0:2, :], wp_r[:, 0:2, :])
            nc.scalar.dma_start(wp[:, 2:4, :], wp_r[:, 2:4, :])
            nc.vector.memset(va[:, :, :, DH:DH + 1], 1.0)

            # ---- filler: split B/proj work into ~0.9us pieces ----
            done = set()
            fq = deque()
            bslot = {}

            def qk_piece(sb, which, ft, half):
                w_sb, dst, bias = ((wq, qt, bqs) if which == 'q'
                                   else (wk, kt, bks))
                key = (which, sb, ft)
                if half == 0:
                    bslot[key] = ps_pair.tile([128, 1024], F32, space="PSUM",
                                              tag="pair", name="pst")
                ps = bslot[key][:, 0:512]
                for do in (range(0, 4) if half == 0 else range(4, 8)):
                    nc.tensor.matmul(
                        ps, w_sb[:, do, ft * 128:(ft + 1) * 128],
                        xtiles[sb][:, do, :], start=(do == 0), stop=(do == 7))
                if half == 1:
                    nc.vector.tensor_scalar_add(
                        dst[:, ft, sb * 512:(sb + 1) * 512], ps,
                        bias[:, ft:ft + 1])
                    del bslot[key]

            def v_piece(sb, st, half):
                key = ('v', sb, st)
                if half == 0:
                    bslot[key] = ps_pair.tile([128, 1024], F32, space="PSUM",
                                              tag="pair", name="pst")
                ps = bslot[key][:, 0:512]
                for do in (range(0, 4) if half == 0 else range(4, 8)):
                    nc.tensor.matmul(
                        ps, xtiles[sb][:, do, st * 128:(st + 1) * 128],
                        wv[:, do, :], start=(do == 0), stop=(do == 7))
                if half == 1:
                    nc.vector.tensor_copy(
                        va[:, sb * 4 + st, :, :DH],
                        ps.rearrange("p (h d) -> p h d", h=8))
                    del bslot[key]

            def proj_piece(qb, jt):
                pst = ps_pair.tile([128, 1024], F32, space="PSUM", tag="pair")
                psj = pst[:, 0:512]
                for co in range(4):
                    nc.tensor.matmul(
                        psj, wp[:, co, jt * 128:(jt + 1) * 128],
                        on_[:, co, qb * 512:(qb + 1) * 512],
                        start=(co == 0), stop=(co == 3))
                ot_sb = op.tile([128, 512], F32, tag="o")
                if qb == 3:  # tail: ACT is idle, DVE is the tail bottleneck
                    nc.scalar.activation(ot_sb[:], psj, AF.Identity,
                                         bias=bps[:, jt:jt + 1])
                else:
                    nc.vector.tensor_scalar_add(ot_sb[:], psj,
                                                bps[:, jt:jt + 1])
                nc.sync.dma_start(
                    out_d[jt * 128:(jt + 1) * 128, qb * 512:(qb + 1) * 512],
                    ot_sb[:])

            # tail split: the qb=3 projection's co=0..2 partials run during
            # hp3's attention (heads 0..5 already renormed); only the co=3
            # matmul waits for the last renorm.
            def proj3_begin(jt):
                pst = ps_pair.tile([128, 1024], F32, space="PSUM",
                                   tag="pair", name="pst")
                bslot[('p3', jt)] = pst
                for co in range(3):
                    nc.tensor.matmul(
                        pst[:, 0:512], wp[:, co, jt * 128:(jt + 1) * 128],
                        on_[:, co, 1536:2048], start=(co == 0), stop=False)

            def proj3_end(jt):
                psj = bslot.pop(('p3', jt))[:, 0:512]
                nc.tensor.matmul(psj, wp[:, 3, jt * 128:(jt + 1) * 128],
                                 on_[:, 3, 1536:2048], start=False, stop=True)
                ot_sb = op.tile([128, 512], F32, tag="o")
                # tail bias-add on ACT (idle at the tail; DVE is busy with
                # the last renorms)
                nc.scalar.activation(ot_sb[:], psj, AF.Identity,
                                     bias=bps[:, jt:jt + 1])
                nc.sync.dma_start(out_d[jt * 128:(jt + 1) * 128, 1536:2048],
                                  ot_sb[:])

            def add_qk(sb, ft):
                for which in ('q', 'k'):
                    fq.append((None, lambda s=sb, w=which, f=ft:
                               qk_piece(s, w, f, 0)))
                    fq.append(((which, sb, ft), lambda s=sb, w=which, f=ft:
                               qk_piece(s, w, f, 1)))

            def add_v(sb):
                for st in range(4):
                    fq.append((None, lambda s=sb, t=st: v_piece(s, t, 0)))
                    fq.append((('v', sb) if st == 3 else None,
                               lambda s=sb, t=st: v_piece(s, t, 1)))

            def pop_emit(n):
                for _ in range(n):
                    if not fq:
                        return
                    key, fn = fq.popleft()
                    fn()
                    if key is not None:
                        done.add(key)

            def drain_until(keys):
                while not all(k in done for k in keys):
                    assert fq, f"filler empty but need {keys}"
                    pop_emit(1)

            # ---- bf16 score matmul for head h, k-tile t, q-block qb ----
            def mm_score(out_ap, h, t, qb, c0, start, stop):
                g2, j = h % 2, h // 2
                p0 = 64 * g2
                lhsT = kt[p0:p0 + 64, j, 128 * t:128 * t + 128]
                rhs = qt[p0:p0 + 64, j, qb * 512 + c0:(qb + 1) * 512]
                nc.tensor.matmul(out_ap, lhsT, rhs, start=start, stop=stop)

            # causal mask on DVE: zero the above-diagonal part of one
            # 128-col diagonal strip of an exp'd half-tile (emitted right
            # after that half's exp so the attn@V dependency is short)
            def emit_mask_strip(et, m, hsl):
                base = et[:, hsl + 128 * m:hsl + 128 * m + 128]
                nc.vector.tensor_tensor(base, base, ltB2[:, 0:128], ALU.mult)

            # ---- attention for (q-block qb, head h) ----
            def emit_av(ot, qb, h, et, u, nt):
                for half in range(2):
                    t = 2 * u + half
                    m = t - 4 * qb
                    c0 = 0 if m < 0 else 128 * m
                    hsl = 512 * half
                    nc.tensor.matmul(
                        ot[0:DH + 1, c0:512], va[:, t, h, :],
                        et[:, hsl + c0:hsl + 512],
                        start=(t == 0), stop=(t == nt - 1),
                        skip_group_check=True)

            def gen_c(qb, h):
                nt = 4 * qb + 4
                qsl = slice(qb * 512, (qb + 1) * 512)
                ot = None  # allocated lazily at first attn@V (avoids a
                # boundary stall on the ot-pool WAR with the previous
                # head-pair's pending renorm)
                prev = None  # (et, u) whose attn@V is deferred one round
                for u in range(nt // 2):
                    pt = ps_pair.tile([128, 1024], F32, space="PSUM", tag="pair")
                    et = ep.tile([128, 1024], BF16, tag="e")
                    m0 = 2 * u - 4 * qb
                    for half in range(2):
                        t = 2 * u + half
                        m = t - 4 * qb
                        hsl = 512 * half
                        c0 = 0 if m < 0 else 128 * m
                        mm_score(pt[:, hsl + c0:hsl + 512], h, t, qb, c0,
                                 True, True)
                    yield
                    # exp (ACT), batched over the pair when both halves full
                    if m0 < 0:
                        nc.scalar.activation(et[:], pt[:], AF.Exp)
                    else:
                        c0a, c0b = 128 * m0, 128 * (m0 + 1)
                        nc.scalar.activation(
                            et[:, c0a:512], pt[:, c0a:512], AF.Exp)
                        emit_mask_strip(et, m0, 0)
                        nc.scalar.activation(
                            et[:, 512 + c0b:1024], pt[:, 512 + c0b:1024],
                            AF.Exp)
                        emit_mask_strip(et, m0 + 1, 512)
                    if prev is not None:
                        if ot is None:
                            ot = ps_ot.tile([DH + 1, 512], F32, space="PSUM",
                                            tag="ot")
                        emit_av(ot, qb, h, prev[0], prev[1], nt)
                    prev = (et, u)
                    yield
                if ot is None:
                    ot = ps_ot.tile([DH + 1, 512], F32, space="PSUM",
                                    tag="ot")
                emit_av(ot, qb, h, prev[0], prev[1], nt)
                # softmax renorm: divide by ones-column row of ot
                dn = sp.tile([1, 512], F32, tag="dn")
                nc.vector.tensor_copy(dn[:], ot[DH:DH + 1, :])
                rec = sp.tile([1, 512], F32, tag="rec")
                nc.vector.reciprocal_approx_fast(rec[:], dn[:])
                rb = sp.tile([DH, 512], F32, tag="rb")
                nc.gpsimd.partition_broadcast(rb[:], rec[:])
                r0 = 64 * (h % 2)
                dst = on_[r0:r0 + 64, h // 2, qsl]
                nc.vector.tensor_tensor(dst, ot[0:DH, :], rb[:], ALU.mult)

            slot = {'i': 0, 'skip': 0, 'every': 1, 'pop': 1}

            def emit_c_pair(qb, h0, h1):
                gens = [gen_c(qb, h0), gen_c(qb, h1)]
                alive = [True, True]
                step = 0
                while any(alive):
                    for i in (0, 1):
                        if alive[i]:
                            try:
                                next(gens[i])
                            except StopIteration:
                                alive[i] = False
                    step += 1
                    if step % 2 == 0:
                        slot['i'] += 1
                        if (slot['i'] > slot['skip']
                                and (slot['i'] - slot['skip']) % slot['every'] == 0):
                            pop_emit(slot['pop'])

            # ---- emission schedule ----
            # critical prefix of B(0): all four Q chains first (bridges PE
            # over the wk/wv DMA arrival), then K ft0 and V
            for ft in range(4):
                qk_piece(0, 'q', ft, 0); qk_piece(0, 'q', ft, 1)
                done.add(('q', 0, ft))
            qk_piece(0, 'k', 0, 0); qk_piece(0, 'k', 0, 1)
            done.add(('k', 0, 0))
            for st in range(4):
                v_piece(0, st, 0); v_piece(0, st, 1)
            done.add(('v', 0))
            for ft in range(1, 4):
                fq.append((None, lambda f=ft: qk_piece(0, 'k', f, 0)))
                fq.append((('k', 0, ft), lambda f=ft: qk_piece(0, 'k', f, 1)))

            # injection config per qb: skip slots after the qb boundary so
            # injected pieces never stall the in-order PE queue on fresh
            # renorm/DMA deps
            inject = {0: (1, 1, 3), 1: (2, 1, 2), 2: (2, 1, 2), 3: (2, 3, 1)}
            for qb in range(4):
                slot['i'] = 0
                slot['skip'], slot['every'], slot['pop'] = inject[qb]
                if qb >= 1:  # proj pieces first: safe once prev qb renormed
                    for jt in range(8):
                        fq.append((None, lambda q=qb - 1, j=jt:
                                   proj_piece(q, j)))
                if qb < 3:
                    emit_xdma(qb + 1)
                    for ft in range(4):
                        add_qk(qb + 1, ft)
                        if ft == 0:
                            add_v(qb + 1)
                for hp in range(4):
                    need = [('q', qb, hp), ('k', qb, hp)]
                    if hp == 0:
                        need.append(('v', qb))
                    if qb == 0 and hp < 3:
                        # prefetch next head-pair's K chain so its DVE
                        # bias-add isn't on the hp-boundary critical path
                        need.append(('k', 0, hp + 1))
                    drain_until(need)
                    if qb == 3 and hp == 3:
                        fq.append((None, lambda: proj3_begin(0)))
                        fq.append((None, lambda: proj3_begin(1)))
                    emit_c_pair(qb, 2 * hp, 2 * hp + 1)
            pop_emit(len(fq))
            proj3_end(0)
            proj3_end(1)
            for jt in range(2, 8):
                proj_piece(3, jt)

    nc.compile()
    return nc


def kernel(x, Wq, bq, Wk, bk, Wv, bv, Wp, bp):
    global _NC
    if _NC is None:
        _NC = _build()

    x = np.asarray(x, np.float32)
    Wq, bq = np.asarray(Wq, np.float32), np.asarray(bq, np.float32)
    Wk, bk = np.asarray(Wk, np.float32), np.asarray(bk, np.float32)
    Wv, bv = np.asarray(Wv, np.float32), np.asarray(bv, np.float32)
    Wp, bp = np.asarray(Wp, np.float32), np.asarray(bp, np.float32)

    bf = ml_dtypes.bfloat16
    i_ = np.arange(128)
    # keep-mask for the DVE multiply: strip element [k_row, q_col] survives
    # iff q >= k within the diagonal 128x128 strip
    keep = (i_[None, :] >= i_[:, None]).astype(np.float32).astype(bf)
    ltB2 = np.ascontiguousarray(np.concatenate([keep, keep], axis=1))

    xbT = [np.ascontiguousarray(x[b].T.astype(bf)) for b in range(B)]

    in_maps = []
    for c in range(N_CORES):
        b, g = c // 2, c % 2
        hs = range(8 * g, 8 * g + 8)
        kidx = np.array([dh * 16 + h for h in hs for dh in range(DH)])
        fsl = slice(FG * g, FG * (g + 1))
        bp_c = (bp if g == 0 else 0.0) + Wp[:, fsl] @ bv[fsl]
        in_maps.append({
            "xbT": xbT[b],
            "wqT": np.ascontiguousarray((SCALE * Wq[fsl].T).astype(bf)),
            "wkT": np.ascontiguousarray(Wk[kidx].T.astype(bf)),
            "wvT": np.ascontiguousarray(Wv[fsl].T.astype(bf)),
            "wpT": np.ascontiguousarray(Wp[:, fsl].T.astype(bf)),
            "bqs": np.ascontiguousarray((SCALE * bq[fsl]).reshape(4, 128).T),
            "bks": np.ascontiguousarray(bk[kidx].reshape(4, 128).T),
            "bps": np.ascontiguousarray(bp_c.reshape(8, 128).T.astype(np.float32)),
            "ltB2": ltB2,
        })

    res = run_bass_kernel_spmd(_NC, in_maps, core_ids=list(range(N_CORES)))
    out = np.empty((B, S, D), np.float32)
    for b in range(B):
        acc = res.results[2 * b]["outT"] + res.results[2 * b + 1]["outT"]
        out[b] = acc.T
    return out
